# revision 51
# baseline (speedup 1.0000x reference)
"""DAM (dist-attention-module) Trainium2 kernel.

Computation (per batch row b):
  cov_temp[b,t]    = relu((x[b,t,:]. xs[b,t,:] - sx*sxs/D) / T)           [B,T]
  cov_channel[b,d] = relu((sum_t x*xs - (sum_t x)(sum_t xs)/T) / D)       [B,D]
  dist = cov_temp outer cov_channel                                       [B,T,D]
  h = LSTM(dist)  (pytorch gate order i,f,g,o)                            [B,T,H]
  spatial  = softmax_t(h)
  temporal = sigmoid(tanh(h) @ attn_W.T + attn_b)

Structure exploited:
 - x_gates = dist @ W_ih.T = ct[:,t] * (cov_channel @ W_ih.T), so the input
   projection is ONE matmul (proj); the per-step contribution is a diagonal
   matmul diag(ct_t) @ proj folded into the recurrent PSUM accumulation,
   and the LSTM bias rides a K=1 ones x bias matmul in the same group.
 - tanh(x) = 2*sigmoid(2x) - 1 everywhere, so the whole scan uses a single
   ACT table set (no per-step table reloads); the temporal-attention dot
   absorbs the affine into its accumulation and host-adjusted bias.

Sharding: pure data parallel, batch 1024 -> 8 cores x 128 rows. Weights and
x/xs in bf16 (f32 accumulators via accum_out), state c in f32.
"""

import numpy as np
import ml_dtypes

import bass_rust as _bass_rust
import concourse.bass as bass
import concourse.bacc as bacc
import concourse.tile as tile
from concourse import mybir
from concourse.bass_utils import run_bass_kernel_spmd

AF = mybir.ActivationFunctionType
OP = mybir.AluOpType

B_FULL = 1024
T = 7
D = 1024
H = 1024
G = 4 * H
P = 128
BC = B_FULL // 8  # batch rows per core = 128
KT = D // P       # 8 k-tiles of 128
NCH = G // 512    # 8 n-chunks of 512
F32 = mybir.dt.float32
BF16 = mybir.dt.bfloat16

_CACHE = {}


def _build_nc():
    nc = bacc.Bacc()

    x_d = nc.dram_tensor("x", [BC, T, D], BF16, kind="ExternalInput")
    xs_d = nc.dram_tensor("xs", [BC, T, D], BF16, kind="ExternalInput")
    wih_d = nc.dram_tensor("wih", [D, G], BF16, kind="ExternalInput")   # W_ih^T
    whh_d = nc.dram_tensor("whh", [D, G], BF16, kind="ExternalInput")   # W_hh^T
    bias_d = nc.dram_tensor("bias4", [1, G], BF16, kind="ExternalInput")  # b_ih+b_hh
    ones_d = nc.dram_tensor("ones", [1, P], BF16, kind="ExternalInput")
    ident_d = nc.dram_tensor("ident", [P, P], BF16, kind="ExternalInput")
    attnw_d = nc.dram_tensor("attnw", [P, D], BF16, kind="ExternalInput")
    attnb_d = nc.dram_tensor("attnb", [P, 1], F32, kind="ExternalInput")

    # spatial output host-side layout [T*BC, D] (t-major) for contiguous stores
    spatial_d = nc.dram_tensor("spatial", [T * BC, D], F32,
                               kind="ExternalOutput")
    temporal_d = nc.dram_tensor("temporal", [BC, T], F32, kind="ExternalOutput")

    with tile.TileContext(nc) as tc:
        # pools are a stack (LIFO close order). whh+wih live low; stage A's
        # small bf16 tiles sit on top and are reused by the scan pools.
        const_cm = tc.tile_pool(name="const", bufs=1)
        persist_cm = tc.tile_pool(name="persist", bufs=1)
        cov_cm = tc.tile_pool(name="cov", bufs=1)
        whhp_cm = tc.tile_pool(name="whhp", bufs=1)
        wihp_cm = tc.tile_pool(name="wihp", bufs=1)
        stage_cm = tc.tile_pool(name="stage", bufs=1)

        const = const_cm.__enter__()
        persist = persist_cm.__enter__()
        cov = cov_cm.__enter__()
        whhp = whhp_cm.__enter__()
        wihp = wihp_cm.__enter__()
        stage = stage_cm.__enter__()

        # ---- stage A inputs first: big transfers head both DMA tracks ----
        x_all = stage.tile([P, T, D], BF16, tag="x_all")
        xs_all = stage.tile([P, T, D], BF16, tag="xs_all")
        nc.sync.dma_start(out=x_all, in_=x_d[:, :, :])
        nc.gpsimd.dma_start(out=xs_all, in_=xs_d[:, :, :])

        # ---- weights: wih needed first (proj), whh by scan t=1 ----
        wih_sb = []
        for k in range(KT):
            w = wihp.tile([P, G], BF16, tag=f"wih{k}", name=f"wih_sb{k}")
            eng = nc.sync if k % 2 == 0 else nc.scalar
            eng.dma_start(out=w, in_=wih_d[k * P:(k + 1) * P, :])
            wih_sb.append(w)

        # ---- constants ----
        ident_sb = const.tile([P, P], BF16, tag="ident")
        nc.sync.dma_start(out=ident_sb, in_=ident_d[:, :])
        ones_sb = const.tile([1, P], BF16, tag="ones")
        nc.sync.dma_start(out=ones_sb, in_=ones_d[:, :])
        attnb_sb = const.tile([P, 1], F32, tag="attnb")
        nc.sync.dma_start(out=attnb_sb, in_=attnb_d[:, :])
        bias_sb = cov.tile([1, G], BF16, tag="bias")
        nc.sync.dma_start(out=bias_sb, in_=bias_d[:, :])
        # whh tiles declared here; DMAs are traced after stage A so the
        # Pool/SP tracks serve stage A first (whh only needed from t=1)
        whh_sb = []
        for k in range(KT):
            w = whhp.tile([P, G], BF16, tag=f"whh{k}", name=f"whh_sb{k}")
            whh_sb.append(w)

        # ---- persistent state ----
        proj_bf = persist.tile([P, G], BF16, tag="proj")
        hT_all = persist.tile([P, KT, P], BF16, tag="hT")
        c_sb = persist.tile([P, H], F32, tag="c")
        ct_sb = persist.tile([P, 8], F32, tag="ct")        # cov_temp
        z_sb = persist.tile([P, 8], F32, tag="z")          # temporal preacts
        tsig = persist.tile([P, 8], F32, tag="tsig")       # temporal out stage

        # ---- cov outputs (outlive the stage pool) ----
        dotp = cov.tile([P, 8], F32, tag="dotp")
        sxc = cov.tile([P, 8], F32, tag="sxc")
        sxsc = cov.tile([P, 8], F32, tag="sxsc")
        covT = cov.tile([P, KT, P], BF16, tag="covT")

        # ---- stage A: covariance statistics (bf16 data, f32 accum_out) ----
        p_sum = stage.tile([P, D], BF16, tag="p_sum")
        x_sum = stage.tile([P, D], BF16, tag="x_sum")
        xs_sum = stage.tile([P, D], BF16, tag="xs_sum")

        pt_all = stage.tile([P, T, D], BF16, tag="pt_all")

        def tree7(dst, base, eng_a, eng_b, nm):
            # dst <- sum_t base[:, t, :] (pairwise, split over two engines)
            a01 = stage.tile([P, D], BF16, tag="a01", name=f"a01_{nm}")
            a23 = stage.tile([P, D], BF16, tag="a23", name=f"a23_{nm}")
            eng_a.tensor_add(a01, base[:, 0, :], base[:, 1, :])
            eng_b.tensor_add(a23, base[:, 2, :], base[:, 3, :])
            eng_a.tensor_add(a01, a01, base[:, 4, :])
            eng_b.tensor_add(a23, a23, base[:, 5, :])
            eng_a.tensor_add(a01, a01, base[:, 6, :])
            eng_a.tensor_add(dst, a01, a23)

        tree7(x_sum, x_all, nc.vector, nc.gpsimd, "x")
        tree7(xs_sum, xs_all, nc.gpsimd, nc.vector, "xs")
        # products as plain muls (bf16 2x mode, ~2x faster than the
        # accum_out STT form); per-t dots ride Pool copies off-path
        for t in range(T):
            nc.vector.tensor_mul(
                pt_all[:, t, :], x_all[:, t, :], xs_all[:, t, :])
        tree7(p_sum, pt_all, nc.vector, nc.gpsimd, "p")
        # ---- cov_channel -> covT (transposed, bf16) ----
        nc.vector.tensor_mul(xs_sum, x_sum, xs_sum)
        nc.vector.scalar_tensor_tensor(
            out=p_sum, in0=xs_sum, scalar=-1.0 / T, in1=p_sum,
            op0=OP.mult, op1=OP.add)
        # whh loads traced here (late). Low-k chunks (needed first by the
        # t=1 recurrent loop) ride the earliest-free tracks; Pool takes two
        # after its stage-A compute drains.
        whh_engs = [nc.gpsimd, nc.gpsimd, nc.sync, nc.sync,
                    nc.sync, nc.sync, nc.scalar, nc.scalar]
        for k in range(KT):
            whh_engs[k].dma_start(
                out=whh_sb[k], in_=whh_d[k * P:(k + 1) * P, :])

        covch = stage.tile([P, D], BF16, tag="covch")
        nc.vector.tensor_scalar(
            out=covch, in0=p_sum, scalar1=0.0, scalar2=1.0 / D,
            op0=OP.max, op1=OP.mult)

        # per-t dots via DVE 4x-mode copies, emitted after the covch-path
        # ops so they trail them in the DVE queue (dotp needed at scan start)
        dve_scr = stage.tile([P, D], BF16, tag="a23", name="dve_scr")
        for t in range(T):
            nc.vector.tensor_scalar(
                out=dve_scr, in0=pt_all[:, t, :], scalar1=1.0, scalar2=0.0,
                op0=OP.mult, op1=OP.add, accum_out=dotp[:, t:t + 1])
        # per-t scalar sums ride ACT copies (off the covch critical path --
        # they only feed cov_temp, which is needed at scan start).
        act_scr = stage.tile([P, D], BF16, tag="a01", name="act_scr")
        for t in range(T):
            nc.scalar.activation(
                out=act_scr, in_=x_all[:, t, :], func=AF.Copy,
                accum_out=sxc[:, t:t + 1])
            nc.scalar.activation(
                out=act_scr, in_=xs_all[:, t, :], func=AF.Copy,
                accum_out=sxsc[:, t:t + 1])

        # ---- cov_temp [P, 7] ----
        corr = cov.tile([P, 8], F32, tag="corr")
        nc.vector.tensor_mul(corr[:, 0:7], sxc[:, 0:7], sxsc[:, 0:7])
        ctp = cov.tile([P, 8], F32, tag="ctp")
        nc.vector.scalar_tensor_tensor(
            out=ctp[:, 0:7], in0=corr[:, 0:7], scalar=-1.0 / D,
            in1=dotp[:, 0:7], op0=OP.mult, op1=OP.add)
        nc.vector.tensor_scalar(
            out=ct_sb[:, 0:7], in0=ctp[:, 0:7], scalar1=0.0, scalar2=1.0 / T,
            op0=OP.max, op1=OP.mult)

        with tc.tile_pool(name="psA", bufs=2, space="PSUM") as psA:
            for k in range(KT):
                pst = psA.tile([P, P], BF16, tag="covtr")
                nc.tensor.transpose(pst, covch[:, k * P:(k + 1) * P], ident_sb)
                nc.vector.tensor_copy(covT[:, k, :], pst)

        stage_cm.__exit__(None, None, None)

        # ---- proj = cov_channel @ W_ih^T  [P, G] bf16 ----
        with tc.tile_pool(name="psPJ", bufs=4, space="PSUM") as psPJ:
            for n in range(NCH):
                ps = psPJ.tile([P, 512], F32, tag="pj")
                for k in range(KT):
                    nc.tensor.matmul(
                        ps, covT[:, k, :], wih_sb[k][:, n * 512:(n + 1) * 512],
                        start=(k == 0), stop=(k == KT - 1))
                nc.vector.tensor_copy(proj_bf[:, n * 512:(n + 1) * 512], ps)


        # W_ih released; scan pools reuse its (and stage A's) space
        wihp_cm.__exit__(None, None, None)

        # =========== LSTM scan + attention folds ======================
        # all transcendentals are Sigmoid (tanh via 2*sigmoid(2x)-1) so the
        # ACT table set is loaded exactly once for the whole scan.
        with tc.tile_pool(name="scan", bufs=2) as sp, \
             tc.tile_pool(name="tail", bufs=1) as tp, \
             tc.tile_pool(name="oio", bufs=5) as oio, \
             tc.tile_pool(name="psG", bufs=6, space="PSUM") as psG, \
             tc.tile_pool(name="psT", bufs=2, space="PSUM") as psT:

            # scan-era residents live in space reclaimed from W_ih/stage A
            h_all = tp.tile([P, T, D], BF16, tag="h_all")
            attnw_sb = tp.tile([P, D], BF16, tag="attnw")
            nc.scalar.dma_start(out=attnw_sb, in_=attnw_d[:, :])

            for t in range(T):
                diag = sp.tile([P, P], BF16, tag="diag")
                nc.vector.tensor_scalar_mul(
                    out=diag, in0=ident_sb, scalar1=ct_sb[:, t:t + 1])

                gates = sp.tile([P, 4, H], BF16, tag="gates")
                # stationary-outer over 4-chunk halves: each stationary
                # (ones, diag, hT_k) is loaded once per half instead of
                # once per chunk -- 4x fewer LDWEIGHTS on the PE
                for gh in range(2):
                    ns = [gh * 4 + j for j in range(4)]
                    pss = [psG.tile([P, 512], F32, tag="g",
                                    name=f"g_{t}_{gh}_{j}") for j in range(4)]
                    for j, n in enumerate(ns):
                        nc.tensor.matmul(
                            pss[j], ones_sb, bias_sb[:, n * 512:(n + 1) * 512],
                            start=True, stop=False, skip_group_check=True)
                    for j, n in enumerate(ns):
                        nc.tensor.matmul(
                            pss[j], diag, proj_bf[:, n * 512:(n + 1) * 512],
                            start=False, stop=(t == 0), skip_group_check=True)
                    if t > 0:
                        for k in range(KT):
                            for j, n in enumerate(ns):
                                nc.tensor.matmul(
                                    pss[j], hT_all[:, k, :],
                                    whh_sb[k][:, n * 512:(n + 1) * 512],
                                    start=False, stop=(k == KT - 1),
                                    skip_group_check=True)
                    for j, n in enumerate(ns):
                        gi, half = n // 2, n % 2
                        # g-gate: sigmoid(2x) now, affine fixup below
                        scale = 2.0 if gi == 2 else 1.0
                        sig_inst = nc.scalar.activation(
                            out=gates[:, gi, half * 512:(half + 1) * 512],
                            in_=pss[j], func=AF.Sigmoid, scale=scale)
                        if t == 6 and gh == 0 and j == 0:
                            sig6_first = sig_inst

                # c = f*c + i*(2*sg-1) ; h = o*(2*sigmoid(2c)-1)
                for hf in range(2):
                    sl = slice(hf * 512, (hf + 1) * 512)
                    nc.vector.tensor_scalar(
                        out=gates[:, 2, sl], in0=gates[:, 2, sl],
                        scalar1=2.0, scalar2=-1.0, op0=OP.mult, op1=OP.add)
                    t1 = sp.tile([P, 512], BF16, tag="t1")
                    nc.vector.tensor_mul(t1, gates[:, 0, sl], gates[:, 2, sl])
                    if t > 0:
                        t2 = sp.tile([P, 512], F32, tag="t2")
                        nc.gpsimd.tensor_mul(t2, gates[:, 1, sl], c_sb[:, sl])
                        nc.vector.tensor_add(c_sb[:, sl], t1, t2)
                    else:
                        nc.vector.tensor_copy(c_sb[:, sl], t1)
                    tanhc = sp.tile([P, 512], BF16, tag="tanhc")
                    nc.scalar.activation(
                        tanhc, c_sb[:, sl], func=AF.Sigmoid, scale=2.0)
                    nc.vector.tensor_scalar(
                        out=tanhc, in0=tanhc,
                        scalar1=2.0, scalar2=-1.0, op0=OP.mult, op1=OP.add)
                    nc.vector.tensor_mul(
                        h_all[:, t, sl], gates[:, 3, sl], tanhc)

                # temporal fold: z[:,t] = sum_d 2*sigmoid(2 h_t)*attn_W
                # (the -sum(attn_W) constant is folded into the host bias)
                th = sp.tile([P, D], BF16, tag="th")
                th_inst = nc.scalar.activation(
                    th, h_all[:, t, :], func=AF.Sigmoid, scale=2.0)
                if t == 5:
                    th5_inst = th_inst
                if t == 6:
                    th6_inst = th_inst
                zscr = sp.tile([P, D], BF16, tag="zscr")
                nc.vector.scalar_tensor_tensor(
                    out=zscr, in0=th, scalar=2.0, in1=attnw_sb,
                    op0=OP.mult, op1=OP.mult, accum_out=z_sb[:, t:t + 1])

                # transpose h for next step's recurrent matmul
                if t < T - 1:
                    for k in range(KT):
                        pst = psT.tile([P, P], BF16, tag="tr")
                        nc.tensor.transpose(
                            pst, h_all[:, t, k * P:(k + 1) * P], ident_sb)
                        nc.vector.tensor_copy(hT_all[:, k, :], pst)


            # ---- spatial softmax over t ----
            e_all = tp.tile([P, T, D], BF16, tag="e_all")
            s_sb = tp.tile([P, D], F32, tag="s")
            s01 = tp.tile([P, D], BF16, tag="s01")
            s23 = tp.tile([P, D], BF16, tag="s23")
            s45 = tp.tile([P, D], BF16, tag="s45")
            for t in range(T):
                e_inst = nc.scalar.activation(
                    e_all[:, t, :], h_all[:, t, :], func=AF.Exp)
                if t < 6:
                    # hoist into step 6's matmul shadow: after step 5's last
                    # sigmoid, as one block before step 6's first sigmoid
                    # (two extra table switches, but off the critical tail)
                    _bass_rust.add_dep_helper(
                        e_inst.ins, th5_inst.ins, sync=False,
                        reason="exp block after step-5 sigmoids")
                    if t == 5:
                        _bass_rust.add_dep_helper(
                            sig6_first.ins, e_inst.ins, sync=False,
                            reason="step-6 sigmoids after exp block")
                else:
                    _bass_rust.add_dep_helper(
                        e_inst.ins, th6_inst.ins, sync=False,
                        reason="exp(h_6) right after step-6 sigmoids")
                    e6_inst = e_inst
                if t == 1:
                    nc.vector.tensor_add(s01, e_all[:, 0, :], e_all[:, 1, :])
                elif t == 3:
                    nc.gpsimd.tensor_add(s23, e_all[:, 2, :], e_all[:, 3, :])
                elif t == 5:
                    nc.vector.tensor_add(s45, e_all[:, 4, :], e_all[:, 5, :])
                    nc.gpsimd.tensor_add(s01, s01, s23)
                    nc.vector.tensor_add(s45, s45, s01)
                elif t == 6:
                    # single op between exp(h_6) and the reciprocal
                    nc.vector.tensor_add(s_sb, s45, e_all[:, 6, :])
            # temporal output: needs the sigmoid set again (its own switch,
            # but the temporal path is tiny and off the spatial chain)
            tsig_inst = nc.scalar.activation(
                out=tsig[:, 0:7], in_=z_sb[:, 0:7], func=AF.Sigmoid,
                bias=attnb_sb, scale=1.0)
            _bass_rust.add_dep_helper(
                tsig_inst.ins, e6_inst.ins, sync=False,
                reason="temporal sigmoid after all exps")
            nc.gpsimd.dma_start(out=temporal_d[:, :], in_=tsig[:, 0:7])

            r_sb = tp.tile([P, D], F32, tag="r")
            nc.vector.reciprocal_approx_fast(out=r_sb, in_=s_sb)
            for t in range(T):
                ot = oio.tile([P, D], F32, tag="ot")
                if t % 2 == 0:
                    nc.vector.tensor_mul(ot, e_all[:, t, :], r_sb)
                else:
                    nc.gpsimd.tensor_mul(ot, e_all[:, t, :], r_sb)
                eng = nc.sync if t % 2 == 0 else nc.scalar
                eng.dma_start(
                    out=spatial_d[t * BC:(t + 1) * BC, :], in_=ot)

        whhp_cm.__exit__(None, None, None)
        cov_cm.__exit__(None, None, None)
        persist_cm.__exit__(None, None, None)
        const_cm.__exit__(None, None, None)

    nc.finalize()
    return nc


def _get_nc():
    if "nc" not in _CACHE:
        _CACHE["nc"] = _build_nc()
    return _CACHE["nc"]


def _make_in_maps(**inputs):
    bf = ml_dtypes.bfloat16
    x = np.asarray(inputs["x"], dtype=np.float32)
    xs = np.asarray(inputs["x_shift"], dtype=np.float32)
    W_ih = np.asarray(inputs["W_ih"], dtype=np.float32)
    W_hh = np.asarray(inputs["W_hh"], dtype=np.float32)
    b4 = (np.asarray(inputs["b_ih"], dtype=np.float32)
          + np.asarray(inputs["b_hh"], dtype=np.float32))
    attn_W = np.asarray(inputs["attn_W"], dtype=np.float32)
    attn_b = np.asarray(inputs["attn_b"], dtype=np.float32)

    wih_t = np.ascontiguousarray(W_ih.T).astype(bf)          # [D, G]
    whh_t = np.ascontiguousarray(W_hh.T).astype(bf)          # [H, G]
    bias4 = b4.reshape(1, G).astype(bf)
    ones = np.ones((1, P), dtype=bf)
    ident = np.eye(P, dtype=np.float32).astype(bf)
    attnw = np.ascontiguousarray(
        np.broadcast_to(attn_W.reshape(1, D), (P, D))).astype(bf)
    # temporal preact accumulates 2*sigmoid(2h)*W; the -sum(attn_W)
    # constant from tanh = 2*sigmoid - 1 folds into the bias.
    badj = float(attn_b.reshape(-1)[0]) - float(attn_W.sum())
    attnb = np.full((P, 1), badj, dtype=np.float32)

    xb = x.astype(bf)
    xsb = xs.astype(bf)
    in_maps = []
    for i in range(8):
        sl = slice(i * BC, (i + 1) * BC)
        in_maps.append({
            "x": np.ascontiguousarray(xb[sl]),
            "xs": np.ascontiguousarray(xsb[sl]),
            "wih": wih_t, "whh": whh_t, "bias4": bias4,
            "ones": ones, "ident": ident,
            "attnw": attnw, "attnb": attnb,
        })
    return in_maps


def kernel(**inputs):
    nc = _get_nc()
    in_maps = _make_in_maps(**inputs)
    res = run_bass_kernel_spmd(nc, in_maps, core_ids=list(range(8)))
    spatial = np.concatenate(
        [r["spatial"].reshape(T, BC, D).transpose(1, 0, 2)
         for r in res.results], axis=0)
    temporal = np.concatenate(
        [r["temporal"] for r in res.results], axis=0).reshape(B_FULL, T, 1)
    return spatial, temporal
